# revision 10
# baseline (speedup 1.0000x reference)
"""Cross-attention block on 8 Trainium2 NeuronCores.

Computes, per batch b:
    xn = LN(x); cn = LN(cond)
    q = xn @ Wq; k = cn @ Wk; v = cn @ Wv   (8 heads x 64)
    out = softmax(q k^T / sqrt(64)) v
    y  = LN(out @ Wo + bo + x)

Sharding: 8 cores = 4 batches x 2 query-row halves (data parallel over
(batch, query-block)).  Each core recomputes LN(cond)/K/V for its batch
and produces a disjoint [1024, 512] slice of the output; no collectives.

v3 design (vs v2 at 283us).  Phase B was exp-bound (ACT pegged at 1.0
for 155us) and phase A added 100us of mostly-serial front time, so:
 - Single software-pipelined stream: attention blocks are per head PAIR
   over all 1024 queries (score matmuls at N=1024, one full bf16 PSUM
   bank per head).  Remaining K/V/Q projections, LN groups, and Wo
   passes interleave under the exp stream instead of running as a
   separate phase.
 - exp ACTIVATEs cover [P, 2, 1024] (2048 elems) and write fp8e4
   directly (bias=-1 for range headroom); ~2.0us/instr vs 2x1.15.
 - attn@V runs as fp8 DoubleRow matmuls: V is stored interleaved over
   key-chunk pairs ([P, mcpair, 2, H, 80] with a ones column at 64 for
   the fused softmax denominator), so one matmul contracts 256 keys.
 - rstd for every LayerNorm = Exp(-0.5 * Ln(var + eps)) on ACT: Ln and
   Exp share one table set, so the exp stream never pays a table switch.
 - All PSUM evacuation on Vector; cen on GpSimd/Vector; ACT = exp only.
 - Wo accumulates per-head-pair partial products into an SBUF f32
   accumulator (y_sb = x + bo + sum_c OT_c @ Wo_c) so only the last
   head pair's Wo pass is tail latency.
PSUM: score tiles 2x2 banks, 2 fixed attention-out banks, 2 rotating
projection/transpose banks = 8 exactly.
"""

import functools

import numpy as np

B, N, M = 4, 2048, 2048
DQ, DC = 512, 768
H, DH = 8, 64
INNER = H * DH  # 512
P = 128
NQ = N // 2  # query rows per core
EPS = 1e-5
N_CORES = 8

FC_X = DQ // P  # 4 feature chunks of x
FC_C = DC // P  # 6 feature chunks of cond
IC = INNER // P  # 4 inner chunks (head pairs)
TQ = NQ // P  # 8 query-token chunks per core
TK = M // P  # 16 key-token chunks
CG = M // 512  # 4 cond token groups
VP = 80  # padded per-head V row (64 vals + den col + pad to 16B)

USE_DR = True  # fp8 DoubleRow for attn@V


def _emit(tc, io):
    import contextlib
    import math

    import concourse.bass as bass
    import concourse.mybir as mybir

    nc = tc.nc
    f32 = mybir.dt.float32
    bf16 = mybir.dt.bfloat16
    f8 = mybir.dt.float8e4
    AF = mybir.ActivationFunctionType
    OP = mybir.AluOpType
    scale = float(DH) ** -0.5

    ctx = contextlib.ExitStack()
    with ctx:
        singles = ctx.enter_context(tc.tile_pool(name="singles", bufs=1))
        work = ctx.enter_context(tc.tile_pool(name="work", bufs=3))
        stat = ctx.enter_context(tc.tile_pool(name="stat", bufs=4))
        cenp = ctx.enter_context(tc.tile_pool(name="cenp", bufs=5))
        ps_st = ctx.enter_context(tc.tile_pool(name="ps_st", bufs=2, space="PSUM"))
        ps_ot = ctx.enter_context(tc.tile_pool(name="ps_ot", bufs=1, space="PSUM"))
        ps_pr = ctx.enter_context(tc.tile_pool(name="ps_pr", bufs=2, space="PSUM"))

        # ---- constants -------------------------------------------------
        from concourse.masks import make_identity

        ident = singles.tile([P, P], bf16, name="ident")
        make_identity(nc, ident)
        eps_t = singles.tile([P, 1], f32, name="eps_t")
        nc.vector.memset(eps_t, EPS)
        neg1_t = singles.tile([P, 1], f32, name="neg1_t")
        nc.vector.memset(neg1_t, -1.0)

        def bcast_load(vec_ap, width, name):
            t = singles.tile([P, width], f32, name=name)
            bc = bass.AP(
                tensor=vec_ap.tensor,
                offset=vec_ap.offset,
                ap=[[0, P]] + [list(a) for a in vec_ap.ap],
            )
            nc.gpsimd.dma_start(out=t, in_=bc)
            return t

        def strip_load(vec_ap, chunks, name):
            t = singles.tile([P, chunks], f32, name=name)
            nc.sync.dma_start(out=t, in_=vec_ap.rearrange("(c p) -> p c", p=P))
            return t

        gx = strip_load(io["lnx_g"], FC_X, "gx")
        bx = strip_load(io["lnx_b"], FC_X, "bx")
        gc = strip_load(io["lnc_g"], FC_C, "gc")
        bc_ = strip_load(io["lnc_b"], FC_C, "bc")

        # ---- weights (bf16, host-cast) ---------------------------------
        wq_b = singles.tile([P, FC_X, INNER], bf16, name="wq_b")
        wk_b = singles.tile([P, FC_C, INNER], bf16, name="wk_b")
        wv_b = singles.tile([P, FC_C, INNER], bf16, name="wv_b")
        wo_b = singles.tile([DH, H, DQ], bf16, name="wo_b")

        # ---- persistent activations ------------------------------------
        xnT = singles.tile([P, FC_X, NQ], bf16, name="xnT")
        cnT = singles.tile([P, FC_C, M], bf16, name="cnT")
        QT = singles.tile([P, IC, NQ], bf16, name="QT")  # (q*scale)^T
        KT = singles.tile([P, IC, M], bf16, name="KT")
        # V interleaved over key-chunk pairs for DoubleRow; ones col at 64.
        V_f8 = singles.tile([P, TK // 2, 2, H, VP], f8, name="V_f8")
        nc.vector.memset(V_f8[:, :, :, :, DH : DH + 1], 1.0)
        # fp8 exp(scores - 1) for one (head-pair, query-half) sub-block:
        # [keys, mc, hslot, q]
        p_blk = singles.tile([P, TK, 2, 512], f8, name="p_blk")
        OT = singles.tile([DH, H, NQ], bf16, name="OT")  # attn out^T
        xres = singles.tile([P, TQ, DQ], f32, name="xres")
        y_sb = singles.tile([P, TQ, DQ], f32, name="y_sb")  # x + bo + Wo acc
        mvs = singles.tile([P, TQ, 2], f32, name="mvs")

        # fixed attention-out accumulators (2 banks, serially reused)
        ot_ab = ps_ot.tile([P, 2, 512], f32, name="ot_ab")

        # ---- LayerNorm group: stats + cen + PE transpose + g/b ---------
        def rstd_of(mv, name):
            # rstd = exp(-0.5 * ln(var + eps)); Ln/Exp share a table set.
            lnv = stat.tile([P, 1], f32, tag="lnv", bufs=4, name=name + "_l")
            nc.scalar.activation(out=lnv, in_=mv[:, 1:2], func=AF.Ln, bias=eps_t)
            r = stat.tile([P, 1], f32, tag="rstd", bufs=4, name=name + "_r")
            nc.scalar.activation(out=r, in_=lnv, func=AF.Exp, scale=-0.5)
            return r

        def ln_group(src, width, tg, g_strip, b_strip, dst, cen_eng):
            fmax = math.gcd(512, width)
            nsub = width // fmax
            fc_n = width // P
            cen_ts = []
            for tl in range(4):
                t = tg * 4 + tl
                if src is None:
                    x_t = xres[:, t]
                else:
                    x_t = work.tile([P, width], f32, tag="xin", bufs=4, name="x_t")
                    nc.sync.dma_start(out=x_t, in_=src[:, t])
                if nsub == 1:
                    stats = stat.tile([P, 6], f32, tag="bnstats", bufs=6, name="st6")
                    nc.vector.bn_stats(out=stats, in_=x_t)
                else:
                    xr_ = x_t.rearrange("p (s f) -> p s f", f=fmax)
                    stats = stat.tile(
                        [P, nsub, 6], f32, tag="bnstats", bufs=6, name="st6"
                    )
                    for s in range(nsub):
                        nc.vector.bn_stats(out=stats[:, s], in_=xr_[:, s])
                mv = stat.tile([P, 2], f32, tag="bnaggr", bufs=6, name="mv")
                nc.vector.bn_aggr(out=mv, in_=stats)
                rstd = rstd_of(mv, "ln")
                nmr = stat.tile([P, 1], f32, tag="nmr", bufs=6, name="nmr")
                nc.vector.scalar_tensor_tensor(
                    out=nmr,
                    in0=mv[:, 0:1],
                    scalar=-1.0,
                    in1=rstd,
                    op0=OP.mult,
                    op1=OP.mult,
                )
                cen = cenp.tile([P, width], bf16, tag="cen", name="cen")
                # (x - mean) * rstd off the ACT engine (exp is sacred)
                cen_eng.tensor_scalar(
                    out=cen,
                    in0=x_t,
                    scalar1=rstd,
                    scalar2=nmr,
                    op0=OP.mult,
                    op1=OP.add,
                )
                cen_ts.append(cen)
            for fc in range(fc_n):
                tp = ps_pr.tile([P, 4, P], bf16, tag="pr", name="tp")
                for tl in range(4):
                    nc.tensor.transpose(
                        tp[:, tl], cen_ts[tl][:, fc * P : (fc + 1) * P], ident
                    )
                nc.vector.tensor_scalar(
                    out=dst[:, fc, tg * 512 : (tg + 1) * 512],
                    in0=tp,
                    scalar1=g_strip[:, fc : fc + 1],
                    scalar2=b_strip[:, fc : fc + 1],
                    op0=OP.mult,
                    op1=OP.add,
                )

        condr = io["cond"].rearrange("(t p) d -> p t d", p=P)
        xr = io["x"].rearrange("(t p) d -> p t d", p=P)

        # ---- projection / Wo / output units (emitted interleaved) ------
        def k_unit(c, g):
            kps = ps_pr.tile([P, 512], f32, tag="pr", name="kps")
            sl = slice(g * 512, (g + 1) * 512)
            for k in range(FC_C):
                nc.tensor.matmul(
                    kps,
                    lhsT=wk_b[:, k, c * P : (c + 1) * P],
                    rhs=cnT[:, k, sl],
                    start=(k == 0),
                    stop=(k == FC_C - 1),
                )
            nc.vector.tensor_copy(out=KT[:, c, sl], in_=kps)

        def q_unit(c, th):
            qps = ps_pr.tile([P, 512], f32, tag="pr", name="qps")
            sl = slice(th * 512, (th + 1) * 512)
            for k in range(FC_X):
                nc.tensor.matmul(
                    qps,
                    lhsT=wq_b[:, k, c * P : (c + 1) * P],
                    rhs=xnT[:, k, sl],
                    start=(k == 0),
                    stop=(k == FC_X - 1),
                )
            nc.vector.tensor_scalar_mul(QT[:, c, sl], qps, scale)

        def v_unit(mh, mc):
            # heads 4*mh .. 4*mh+3 for key chunk mc
            vps = ps_pr.tile([P, 512], f32, tag="pr", name="vps")
            for k in range(FC_C):
                nc.tensor.matmul(
                    vps[:, 0:256],
                    lhsT=cnT[:, k, mc * P : (mc + 1) * P],
                    rhs=wv_b[:, k, mh * 256 : (mh + 1) * 256],
                    start=(k == 0),
                    stop=(k == FC_C - 1),
                )
            nc.vector.tensor_copy(
                out=V_f8[:, mc // 2, mc % 2, 4 * mh : 4 * mh + 4, 0:DH],
                in_=vps[:, 0:256].rearrange("p (h d) -> p h d", d=DH),
            )

        def wo_unit(c, t):
            y_ps = ps_pr.tile([P, 512], f32, tag="pr", name="y_ps")
            for j, h in enumerate((2 * c, 2 * c + 1)):
                nc.tensor.matmul(
                    y_ps,
                    lhsT=OT[:, h, t * P : (t + 1) * P],
                    rhs=wo_b[:, h, :],
                    start=(j == 0),
                    stop=(j == 1),
                )
            nc.vector.tensor_add(out=y_sb[:, t], in0=y_sb[:, t], in1=y_ps)

        outr = io["out"].rearrange("(t p) d -> p t d", p=P)

        def final_unit(t):
            stats = stat.tile([P, 6], f32, tag="bnstats", bufs=6, name="stf")
            nc.vector.bn_stats(out=stats, in_=y_sb[:, t])
            nc.vector.bn_aggr(out=mvs[:, t], in_=stats)
            rstd = rstd_of(mvs[:, t], "fin")
            y1 = y_sb[:, t]
            nc.vector.tensor_scalar(
                out=y1,
                in0=y1,
                scalar1=mvs[:, t, 0:1],
                scalar2=rstd,
                op0=OP.subtract,
                op1=OP.mult,
            )
            nc.vector.tensor_mul(out=y1, in0=y1, in1=gf_bc)
            nc.gpsimd.tensor_add(out=y1, in0=y1, in1=bf_bc)
            nc.sync.dma_start(out=outr[:, t], in_=y1)

        # ---- attention pieces ------------------------------------------
        def score_exp(c, mc, qh):
            sl = slice(qh * 512, (qh + 1) * 512)
            st = ps_st.tile([P, 2, 512], f32, tag="st", name="st")
            nc.tensor.matmul(
                st[:, 0],
                lhsT=KT[0:DH, c, mc * P : (mc + 1) * P],
                rhs=QT[0:DH, c, sl],
                start=True,
                stop=True,
            )
            nc.tensor.matmul(
                st[:, 1],
                lhsT=KT[DH:P, c, mc * P : (mc + 1) * P],
                rhs=QT[DH:P, c, sl],
                start=True,
                stop=True,
            )
            # exp(score - 1): -1 keeps exp(max) inside fp8e4 range.
            nc.scalar.activation(
                out=p_blk[:, mc], in_=st, func=AF.Exp, bias=neg1_t
            )

        def pv_pair(c, mp):
            # attn@V for key chunks (2mp, 2mp+1), both heads of the pair
            for j in range(2):
                if USE_DR:
                    nc.tensor.matmul(
                        ot_ab[0 : DH + 1, j],
                        lhsT=V_f8[:, mp, :, 2 * c + j, 0 : DH + 1],
                        rhs=p_blk[:, 2 * mp : 2 * mp + 2, j, :],
                        start=(mp == 0),
                        stop=(mp == TK // 2 - 1),
                        perf_mode=mybir.MatmulPerfMode.DoubleRow,
                    )
                else:
                    for par in range(2):
                        nc.tensor.matmul(
                            ot_ab[0 : DH + 1, j],
                            lhsT=V_f8[:, mp, par, 2 * c + j, 0 : DH + 1],
                            rhs=p_blk[:, 2 * mp + par, j, :],
                            start=(mp == 0 and par == 0),
                            stop=(mp == TK // 2 - 1 and par == 1),
                        )

        def normalize(c, qh):
            sl = slice(qh * 512, (qh + 1) * 512)
            for j, h in enumerate((2 * c, 2 * c + 1)):
                rb = work.tile([P, 512], f32, tag="rb", bufs=2, name="rb")
                # full-tile approx reciprocal; only row DH (denominator)
                # matters, junk rows are overwritten by the broadcast.
                nc.vector.reciprocal_approx_fast(out=rb, in_=ot_ab[:, j])
                r0 = work.tile([1, 512], f32, tag="r0", bufs=2, name="r0")
                nc.sync.dma_start(out=r0, in_=rb[DH : DH + 1, :])
                nc.gpsimd.partition_broadcast(rb[0:DH, :], r0[0:1, :])
                nc.vector.tensor_mul(
                    out=OT[:, h, sl], in0=ot_ab[0:DH, j], in1=rb[0:DH, :]
                )

        # ================= emission schedule =============================
        # pre-stream: DMAs ordered Wk -> Wq -> x -> cond g0 -> Wv -> cond
        # g1..g3; LN + K/Q chunk 0 + first V units so the exp stream
        # starts ~16us in and stays fed.
        nc.sync.dma_start(out=wk_b, in_=io["Wk"].rearrange("(ko p) i -> p ko i", p=P))
        nc.sync.dma_start(out=wq_b, in_=io["Wq"].rearrange("(ko p) i -> p ko i", p=P))
        for t in range(TQ):
            nc.sync.dma_start(out=xres[:, t], in_=xr[:, t])
        ln_group(condr, DC, 0, gc, bc_, cnT, nc.gpsimd)
        nc.sync.dma_start(out=wv_b, in_=io["Wv"].rearrange("(ko p) i -> p ko i", p=P))
        # exp table preload off the critical path
        dummy = stat.tile([1, 1], f32, tag="dummy", bufs=1, name="dummy")
        nc.scalar.activation(out=dummy, in_=eps_t[0:1, 0:1], func=AF.Exp)
        ln_group(None, DQ, 0, gx, bx, xnT, nc.vector)
        ln_group(None, DQ, 1, gx, bx, xnT, nc.vector)
        k_unit(0, 0)
        q_unit(0, 0)
        q_unit(0, 1)
        for mc in range(4):
            v_unit(0, mc)
        for g in range(1, CG):
            ln_group(condr, DC, g, gc, bc_, cnT, nc.gpsimd)
            k_unit(0, g)
        nc.sync.dma_start(out=wo_b, in_=io["Wo"].rearrange("(h p) d -> p h d", p=DH))
        gf_bc = bcast_load(io["lnf_g"], DQ, "gf_bc")
        bf_bc = bcast_load(io["lnf_b"], DQ, "bf_bc")
        bo_bc = bcast_load(io["bo"], DQ, "bo_bc")
        for t in range(TQ):
            nc.gpsimd.tensor_add(out=y_sb[:, t], in0=xres[:, t], in1=bo_bc)

        # deferred units, emitted right after slot (c, qh, mc)'s
        # score+exp.  8 sub-blocks of 16 exp slots each.
        defer = {
            (c, qh): {mc: [] for mc in range(TK)}
            for c in range(IC)
            for qh in range(2)
        }

        def put(c, qh, mc, fn):
            defer[(c, qh)][mc].append(fn)

        # V half-groups: heads 0-3 during (0,0) (chunks 0-3 pre-stream,
        # JIT ahead of the PV pairs that need them); heads 4-7 (used from
        # c=2) spread over (0,1)/(1,0).
        for mc in range(4, TK):
            put(0, 0, mc - 2, functools.partial(v_unit, 0, mc))
        for mc in range(TK):
            c, qh, slot = (0, 1, 2 * mc) if mc < 8 else (1, 0, 2 * (mc - 8))
            put(c, qh, slot, functools.partial(v_unit, 1, mc))
        # K for chunk c+1: two cond groups per sub-block of block c
        for c in range(IC - 1):
            put(c, 0, 5, functools.partial(k_unit, c + 1, 0))
            put(c, 0, 11, functools.partial(k_unit, c + 1, 1))
            put(c, 1, 5, functools.partial(k_unit, c + 1, 2))
            put(c, 1, 11, functools.partial(k_unit, c + 1, 3))
            put(c, 1, 8, functools.partial(q_unit, c + 1, 0))
            put(c, 1, 14, functools.partial(q_unit, c + 1, 1))
        # Wo pass for block c during sub-block (c+1, 0)
        for c in range(IC - 1):
            for t in range(TQ):
                put(c + 1, 0, 2 * t + 1, functools.partial(wo_unit, c, t))

        for c in range(IC):
            for qh in range(2):
                for mc in range(TK):
                    score_exp(c, mc, qh)
                    for fn in defer[(c, qh)][mc]:
                        fn()
                    if mc % 2 == 1:
                        pv_pair(c, (mc - 1) // 2)
                normalize(c, qh)
        for t in range(TQ):
            wo_unit(IC - 1, t)
            final_unit(t)


@functools.cache
def _build_program():
    import concourse.bacc as bacc
    import concourse.mybir as mybir
    import concourse.tile as tile

    f32 = mybir.dt.float32
    bf16 = mybir.dt.bfloat16
    nc = bacc.Bacc()
    io = {}
    io["x"] = nc.declare_dram_parameter("x", [NQ, DQ], f32, False)[:, :]
    io["cond"] = nc.declare_dram_parameter("cond", [M, DC], f32, False)[:, :]
    for name in ("lnx_g", "lnx_b"):
        io[name] = nc.declare_dram_parameter(name, [DQ], f32, False)[:]
    for name in ("lnc_g", "lnc_b"):
        io[name] = nc.declare_dram_parameter(name, [DC], f32, False)[:]
    io["Wq"] = nc.declare_dram_parameter("Wq", [DQ, INNER], bf16, False)[:, :]
    io["Wk"] = nc.declare_dram_parameter("Wk", [DC, INNER], bf16, False)[:, :]
    io["Wv"] = nc.declare_dram_parameter("Wv", [DC, INNER], bf16, False)[:, :]
    io["Wo"] = nc.declare_dram_parameter("Wo", [INNER, DQ], bf16, False)[:, :]
    for name in ("bo", "lnf_g", "lnf_b"):
        io[name] = nc.declare_dram_parameter(name, [DQ], f32, False)[:]
    io["out"] = nc.declare_dram_parameter("out", [NQ, DQ], f32, True)[:, :]

    with tile.TileContext(nc) as tc:
        _emit(tc, io)
    nc.compile()
    return nc


def _core_input_map(inputs, core):
    import ml_dtypes

    b, half = core // 2, core % 2
    m = {
        "x": np.ascontiguousarray(inputs["x"][b, half * NQ : (half + 1) * NQ]),
        "cond": np.ascontiguousarray(inputs["cond"][b]),
    }
    for name in ("lnx_g", "lnx_b", "lnc_g", "lnc_b", "bo", "lnf_g", "lnf_b"):
        m[name] = np.asarray(inputs[name], dtype=np.float32)
    for name in ("Wq", "Wk", "Wv", "Wo"):
        m[name] = np.asarray(inputs[name]).astype(ml_dtypes.bfloat16)
    return m


TRACE = False
LAST_RESULTS = None


def kernel(**inputs):
    from concourse.bass_utils import run_bass_kernel_spmd

    global LAST_RESULTS
    nc = _build_program()
    in_maps = [_core_input_map(inputs, core) for core in range(N_CORES)]
    res = run_bass_kernel_spmd(
        nc,
        in_maps,
        list(range(N_CORES)),
        trace=TRACE,
        trace_cores=[0] if TRACE else None,
    )
    LAST_RESULTS = res
    out = np.empty((B, N, DQ), np.float32)
    for core in range(N_CORES):
        b, half = core // 2, core % 2
        out[b, half * NQ : (half + 1) * NQ] = res.results[core]["out"]
    return out


# revision 23
# speedup vs baseline: 1.2194x; 1.2194x over previous
"""Cross-attention block on 8 Trainium2 NeuronCores.

Computes, per batch b:
    xn = LN(x); cn = LN(cond)
    q = xn @ Wq; k = cn @ Wk; v = cn @ Wv   (8 heads x 64)
    out = softmax(q k^T / sqrt(64)) v
    y  = LN(out @ Wo + bo + x)

Sharding: 8 cores = 4 batches x 2 query-row halves (data parallel over
(batch, query-block)).  Each core recomputes LN(cond)/K/V for its batch
and produces a disjoint [1024, 512] slice of the output; no collectives.

v3 design (vs v2 at 283us).  Phase B was exp-bound (ACT pegged at 1.0
for 155us) and phase A added 100us of mostly-serial front time, so:
 - Single software-pipelined stream: attention blocks are per head PAIR
   over all 1024 queries (score matmuls at N=1024, one full bf16 PSUM
   bank per head).  Remaining K/V/Q projections, LN groups, and Wo
   passes interleave under the exp stream instead of running as a
   separate phase.
 - exp ACTIVATEs cover [P, 2, 1024] (2048 elems) and write fp8e4
   directly (bias=-1 for range headroom); ~2.0us/instr vs 2x1.15.
 - attn@V runs as fp8 DoubleRow matmuls: V is stored interleaved over
   key-chunk pairs ([P, mcpair, 2, H, 80] with a ones column at 64 for
   the fused softmax denominator), so one matmul contracts 256 keys.
 - rstd for every LayerNorm = Exp(-0.5 * Ln(var + eps)) on ACT: Ln and
   Exp share one table set, so the exp stream never pays a table switch.
 - All PSUM evacuation on Vector; cen on GpSimd/Vector; ACT = exp only.
 - Wo accumulates per-head-pair partial products into an SBUF f32
   accumulator (y_sb = x + bo + sum_c OT_c @ Wo_c) so only the last
   head pair's Wo pass is tail latency.
PSUM: score tiles 2x2 banks, 2 fixed attention-out banks, 2 rotating
projection/transpose banks = 8 exactly.
"""

import functools

import numpy as np

B, N, M = 4, 2048, 2048
DQ, DC = 512, 768
H, DH = 8, 64
INNER = H * DH  # 512
P = 128
NQ = N // 2  # query rows per core
EPS = 1e-5
N_CORES = 8

FC_X = DQ // P  # 4 feature chunks of x
FC_C = DC // P  # 6 feature chunks of cond
IC = INNER // P  # 4 inner chunks (head pairs)
TQ = NQ // P  # 8 query-token chunks per core
TK = M // P  # 16 key-token chunks
CG = M // 512  # 4 cond token groups
VP = 80  # padded per-head V row (64 vals + den col + pad to 16B)

USE_DR = True  # fp8 DoubleRow for attn@V
USE_DR_PROJ = False  # fp8 DoubleRow K/V/Q projections (fp8 cnT/xnT)


def _emit(tc, io):
    import contextlib
    import math

    import concourse.bass as bass
    import concourse.mybir as mybir

    nc = tc.nc
    f32 = mybir.dt.float32
    bf16 = mybir.dt.bfloat16
    f8 = mybir.dt.float8e4
    AF = mybir.ActivationFunctionType
    OP = mybir.AluOpType
    scale = float(DH) ** -0.5

    ctx = contextlib.ExitStack()
    with ctx:
        singles = ctx.enter_context(tc.tile_pool(name="singles", bufs=1))
        work = ctx.enter_context(tc.tile_pool(name="work", bufs=3))
        stat = ctx.enter_context(tc.tile_pool(name="stat", bufs=4))
        cenp = ctx.enter_context(tc.tile_pool(name="cenp", bufs=5))
        ps_st = ctx.enter_context(tc.tile_pool(name="ps_st", bufs=2, space="PSUM"))
        ps_ot = ctx.enter_context(tc.tile_pool(name="ps_ot", bufs=1, space="PSUM"))
        ps_pr = ctx.enter_context(tc.tile_pool(name="ps_pr", bufs=2, space="PSUM"))

        # ---- constants -------------------------------------------------
        from concourse.masks import make_identity

        ident = singles.tile([P, P], bf16, name="ident")
        make_identity(nc, ident)
        identa = ident
        if USE_DR_PROJ:
            identa = singles.tile([P, P], f8, name="identa")
            make_identity(nc, identa)
        eps_t = singles.tile([P, 1], f32, name="eps_t")
        nc.vector.memset(eps_t, EPS)
        neg1_t = singles.tile([P, 1], f32, name="neg1_t")
        nc.vector.memset(neg1_t, -1.0)

        def bcast_load(vec_ap, width, name):
            t = singles.tile([P, width], f32, name=name)
            bc = bass.AP(
                tensor=vec_ap.tensor,
                offset=vec_ap.offset,
                ap=[[0, P]] + [list(a) for a in vec_ap.ap],
            )
            nc.gpsimd.dma_start(out=t, in_=bc)
            return t

        def strip_load(vec_ap, chunks, name):
            t = singles.tile([P, chunks], f32, name=name)
            nc.sync.dma_start(out=t, in_=vec_ap.rearrange("(c p) -> p c", p=P))
            return t

        gx = strip_load(io["lnx_g"], FC_X, "gx")
        bx = strip_load(io["lnx_b"], FC_X, "bx")
        gc = strip_load(io["lnc_g"], FC_C, "gc")
        bc_ = strip_load(io["lnc_b"], FC_C, "bc")

        # ---- weights (host-cast) ---------------------------------------
        adt = f8 if USE_DR_PROJ else bf16  # projection-input dtype
        if USE_DR_PROJ:
            # feature axis interleaved in (chunk, ko, partition) pairs
            wq_b = singles.tile([P, FC_X // 2, 2, INNER], f8, name="wq_b")
            wk_b = singles.tile([P, FC_C // 2, 2, INNER], f8, name="wk_b")
            wv_b = singles.tile([P, FC_C // 2, 2, INNER], f8, name="wv_b")
            wq_src = io["Wq"].rearrange("(c j p) i -> p c j i", j=2, p=P)
            wk_src = io["Wk"].rearrange("(c j p) i -> p c j i", j=2, p=P)
            wv_src = io["Wv"].rearrange("(c j p) i -> p c j i", j=2, p=P)
        else:
            wq_b = singles.tile([P, FC_X, INNER], bf16, name="wq_b")
            wk_b = singles.tile([P, FC_C, INNER], bf16, name="wk_b")
            wv_b = singles.tile([P, FC_C, INNER], bf16, name="wv_b")
            wq_src = io["Wq"].rearrange("(ko p) i -> p ko i", p=P)
            wk_src = io["Wk"].rearrange("(ko p) i -> p ko i", p=P)
            wv_src = io["Wv"].rearrange("(ko p) i -> p ko i", p=P)
        wo_b = singles.tile([DH, H, DQ], bf16, name="wo_b")

        # ---- persistent activations ------------------------------------
        xnT = singles.tile([P, FC_X, NQ], adt, name="xnT")
        cnT = singles.tile([P, FC_C, M], adt, name="cnT")
        QT = singles.tile([P, IC, NQ], bf16, name="QT")  # (q*scale)^T
        KT = singles.tile([P, IC, M], bf16, name="KT")
        # V interleaved over key-chunk pairs for DoubleRow; ones col at 64.
        V_f8 = singles.tile([P, TK // 2, 2, H, VP], f8, name="V_f8")
        nc.vector.memset(V_f8[:, :, :, :, DH : DH + 1], 1.0)
        # fp8 exp(scores - 1) for one (head-pair, query-half) sub-block:
        # [keys, mc, hslot, q]
        p_blk = singles.tile([P, TK, 2, 512], f8, name="p_blk")
        OT = singles.tile([DH, H, NQ], bf16, name="OT")  # attn out^T
        # xres doubles as the output accumulator: after the x LN stats
        # and cen reads it, y = x + bo + sum_c OT_c @ Wo_c is built in place.
        xres = singles.tile([P, TQ, DQ], f32, name="xres")
        y_sb = xres
        mvs = singles.tile([P, TQ, 2], f32, name="mvs")

        # fixed attention-out accumulators (2 banks, serially reused)
        ot_ab = ps_ot.tile([P, 2, 512], f32, name="ot_ab")

        # centered inputs for cond groups 1-3 (deferred PE transposes)
        cen_hold = singles.tile([P, 3, 4, DC], adt, name="cen_hold")

        # ---- LayerNorm pieces ------------------------------------------
        # All rstds use ACT Sqrt, emitted pre-stream (plus one batched
        # group for the final LN in the tail) so the exp table set is
        # loaded exactly once for the whole stream.
        def ln_stats_cen(src, width, tg, dma_eng, cen_eng, cen_dst):
            """DMA + stats + rstd + centered-normalized tiles for one
            512-token group.  cen_dst: list of 4 [P, width] bf16 APs."""
            fmax = math.gcd(512, width)
            nsub = width // fmax
            for tl in range(4):
                t = tg * 4 + tl
                if src is None:
                    x_t = xres[:, t]
                else:
                    x_t = work.tile([P, width], f32, tag="xin", bufs=6, name="x_t")
                    dma_eng.dma_start(out=x_t, in_=src[:, t])
                if nsub == 1:
                    stats = stat.tile([P, 6], f32, tag="bnstats", bufs=6, name="st6")
                    nc.vector.bn_stats(out=stats, in_=x_t)
                else:
                    xr_ = x_t.rearrange("p (s f) -> p s f", f=fmax)
                    stats = stat.tile(
                        [P, nsub, 6], f32, tag="bnstats", bufs=6, name="st6"
                    )
                    for s in range(nsub):
                        nc.vector.bn_stats(out=stats[:, s], in_=xr_[:, s])
                mv = stat.tile([P, 2], f32, tag="bnaggr", bufs=6, name="mv")
                nc.vector.bn_aggr(out=mv, in_=stats)
                std = stat.tile([P, 1], f32, tag="std", bufs=6, name="std")
                nc.scalar.activation(
                    out=std, in_=mv[:, 1:2], func=AF.Sqrt, bias=eps_t
                )
                rstd = stat.tile([P, 1], f32, tag="rstd", bufs=6, name="rstd")
                nc.vector.reciprocal(out=rstd, in_=std)
                nmr = stat.tile([P, 1], f32, tag="nmr", bufs=6, name="nmr")
                nc.vector.scalar_tensor_tensor(
                    out=nmr,
                    in0=mv[:, 0:1],
                    scalar=-1.0,
                    in1=rstd,
                    op0=OP.mult,
                    op1=OP.mult,
                )
                cen_eng.tensor_scalar(
                    out=cen_dst[tl],
                    in0=x_t,
                    scalar1=rstd,
                    scalar2=nmr,
                    op0=OP.mult,
                    op1=OP.add,
                )

        def ln_apply(cen_ts, g_strip, b_strip, dst, tg, fc_lo, fc_hi):
            # PE transpose + gamma/beta for feature chunks [fc_lo, fc_hi)
            for fc in range(fc_lo, fc_hi):
                tp = ps_pr.tile([P, 4, P], adt, tag="pr", name="tp")
                for tl in range(4):
                    nc.tensor.transpose(
                        tp[:, tl], cen_ts[tl][:, fc * P : (fc + 1) * P], identa
                    )
                nc.vector.tensor_scalar(
                    out=dst[:, fc, tg * 512 : (tg + 1) * 512],
                    in0=tp,
                    scalar1=g_strip[:, fc : fc + 1],
                    scalar2=b_strip[:, fc : fc + 1],
                    op0=OP.mult,
                    op1=OP.add,
                )

        condr = io["cond"].rearrange("(t p) d -> p t d", p=P)
        xr = io["x"].rearrange("(t p) d -> p t d", p=P)

        # ---- projection / Wo / output units (emitted interleaved) ------
        DRM = mybir.MatmulPerfMode.DoubleRow

        def k_unit(c, g):
            kps = ps_pr.tile([P, 512], f32, tag="pr", name="kps")
            sl = slice(g * 512, (g + 1) * 512)
            if USE_DR_PROJ:
                for k in range(FC_C // 2):
                    nc.tensor.matmul(
                        kps,
                        lhsT=wk_b[:, k, :, c * P : (c + 1) * P],
                        rhs=cnT[:, 2 * k : 2 * k + 2, sl],
                        start=(k == 0),
                        stop=(k == FC_C // 2 - 1),
                        perf_mode=DRM,
                    )
            else:
                for k in range(FC_C):
                    nc.tensor.matmul(
                        kps,
                        lhsT=wk_b[:, k, c * P : (c + 1) * P],
                        rhs=cnT[:, k, sl],
                        start=(k == 0),
                        stop=(k == FC_C - 1),
                    )
            nc.vector.tensor_copy(out=KT[:, c, sl], in_=kps)

        def q_unit(c, th):
            qps = ps_pr.tile([P, 512], f32, tag="pr", name="qps")
            sl = slice(th * 512, (th + 1) * 512)
            if USE_DR_PROJ:
                for k in range(FC_X // 2):
                    nc.tensor.matmul(
                        qps,
                        lhsT=wq_b[:, k, :, c * P : (c + 1) * P],
                        rhs=xnT[:, 2 * k : 2 * k + 2, sl],
                        start=(k == 0),
                        stop=(k == FC_X // 2 - 1),
                        perf_mode=DRM,
                    )
            else:
                for k in range(FC_X):
                    nc.tensor.matmul(
                        qps,
                        lhsT=wq_b[:, k, c * P : (c + 1) * P],
                        rhs=xnT[:, k, sl],
                        start=(k == 0),
                        stop=(k == FC_X - 1),
                    )
            nc.vector.tensor_scalar_mul(QT[:, c, sl], qps, scale)

        def v_unit(mh, mc):
            # heads 4*mh .. 4*mh+3 for key chunk mc
            vps = ps_pr.tile([P, 512], f32, tag="pr", name="vps")
            msl = slice(mc * P, (mc + 1) * P)
            if USE_DR_PROJ:
                for k in range(FC_C // 2):
                    nc.tensor.matmul(
                        vps[:, 0:256],
                        lhsT=cnT[:, 2 * k : 2 * k + 2, msl],
                        rhs=wv_b[:, k, :, mh * 256 : (mh + 1) * 256],
                        start=(k == 0),
                        stop=(k == FC_C // 2 - 1),
                        perf_mode=DRM,
                    )
            else:
                for k in range(FC_C):
                    nc.tensor.matmul(
                        vps[:, 0:256],
                        lhsT=cnT[:, k, msl],
                        rhs=wv_b[:, k, mh * 256 : (mh + 1) * 256],
                        start=(k == 0),
                        stop=(k == FC_C - 1),
                    )
            nc.vector.tensor_copy(
                out=V_f8[:, mc // 2, mc % 2, 4 * mh : 4 * mh + 4, 0:DH],
                in_=vps[:, 0:256].rearrange("p (h d) -> p h d", d=DH),
            )

        def wo_unit(c, t):
            y_ps = ps_pr.tile([P, 512], f32, tag="pr", name="y_ps")
            for j, h in enumerate((2 * c, 2 * c + 1)):
                nc.tensor.matmul(
                    y_ps,
                    lhsT=OT[:, h, t * P : (t + 1) * P],
                    rhs=wo_b[:, h, :],
                    start=(j == 0),
                    stop=(j == 1),
                )
            nc.vector.tensor_add(out=y_sb[:, t], in0=y_sb[:, t], in1=y_ps)

        outr = io["out"].rearrange("(t p) d -> p t d", p=P)

        # ---- attention pieces ------------------------------------------
        def score_exp(c, mc, qh):
            sl = slice(qh * 512, (qh + 1) * 512)
            st = ps_st.tile([P, 2, 512], f32, tag="st", name="st")
            nc.tensor.matmul(
                st[:, 0],
                lhsT=KT[0:DH, c, mc * P : (mc + 1) * P],
                rhs=QT[0:DH, c, sl],
                start=True,
                stop=True,
            )
            nc.tensor.matmul(
                st[:, 1],
                lhsT=KT[DH:P, c, mc * P : (mc + 1) * P],
                rhs=QT[DH:P, c, sl],
                start=True,
                stop=True,
            )
            # exp(score - 1): -1 keeps exp(max) inside fp8e4 range.
            nc.scalar.activation(
                out=p_blk[:, mc], in_=st, func=AF.Exp, bias=neg1_t
            )

        def pv_pair(c, mp):
            # attn@V for key chunks (2mp, 2mp+1), both heads of the pair
            for j in range(2):
                if USE_DR:
                    nc.tensor.matmul(
                        ot_ab[0 : DH + 1, j],
                        lhsT=V_f8[:, mp, :, 2 * c + j, 0 : DH + 1],
                        rhs=p_blk[:, 2 * mp : 2 * mp + 2, j, :],
                        start=(mp == 0),
                        stop=(mp == TK // 2 - 1),
                        perf_mode=mybir.MatmulPerfMode.DoubleRow,
                    )
                else:
                    for par in range(2):
                        nc.tensor.matmul(
                            ot_ab[0 : DH + 1, j],
                            lhsT=V_f8[:, mp, par, 2 * c + j, 0 : DH + 1],
                            rhs=p_blk[:, 2 * mp + par, j, :],
                            start=(mp == 0 and par == 0),
                            stop=(mp == TK // 2 - 1 and par == 1),
                        )

        def normalize(c, qh):
            sl = slice(qh * 512, (qh + 1) * 512)
            for j, h in enumerate((2 * c, 2 * c + 1)):
                rb = work.tile([P, 512], f32, tag="rb", bufs=2, name="rb")
                # full-tile approx reciprocal; only row DH (denominator)
                # matters, junk rows are overwritten by the broadcast.
                nc.vector.reciprocal_approx_fast(out=rb, in_=ot_ab[:, j])
                r0 = work.tile([1, 512], f32, tag="r0", bufs=2, name="r0")
                nc.gpsimd.dma_start(out=r0, in_=rb[DH : DH + 1, :])
                nc.gpsimd.partition_broadcast(rb[0:DH, :], r0[0:1, :])
                nc.vector.tensor_mul(
                    out=OT[:, h, sl], in0=ot_ab[0:DH, j], in1=rb[0:DH, :]
                )

        # ================= emission schedule =============================
        # Input DMAs spread over four engine queues so they land in
        # parallel; pre-stream PE holds ONLY what the first exp needs
        # (cond g0 + x transposes, K/Q chunk 0).  Everything else runs
        # as deferred units inside the exp stream.
        cen_p = {}  # (kind, group) -> list of 4 cen APs
        cen_p["c", 0] = [
            cenp.tile([P, DC], adt, tag="cen", name="cen0") for _ in range(4)
        ]
        for g in (1, 2, 3):
            cen_p["c", g] = [cen_hold[:, g - 1, tl] for tl in range(4)]
        for g in (0, 1):
            cen_p["x", g] = [
                cenp.tile([P, DQ], adt, tag="cenx", bufs=8, name="cenx")
                for _ in range(4)
            ]
        # One DMA queue, strict priority order: everything shares one
        # ~400GB/s HBM port, so landing ORDER is what matters.
        nc.sync.dma_start(out=wk_b, in_=wk_src)
        ln_stats_cen(condr, DC, 0, nc.sync, nc.vector, cen_p["c", 0])
        nc.sync.dma_start(out=wq_b, in_=wq_src)
        for t in range(4):
            nc.sync.dma_start(out=xres[:, t], in_=xr[:, t])
        ln_stats_cen(None, DQ, 0, None, nc.vector, cen_p["x", 0])
        ln_stats_cen(condr, DC, 1, nc.sync, nc.vector, cen_p["c", 1])
        for t in range(4, TQ):
            nc.sync.dma_start(out=xres[:, t], in_=xr[:, t])
        ln_stats_cen(None, DQ, 1, None, nc.vector, cen_p["x", 1])
        nc.sync.dma_start(out=wv_b, in_=wv_src)
        ln_stats_cen(condr, DC, 2, nc.sync, nc.gpsimd, cen_p["c", 2])
        ln_stats_cen(condr, DC, 3, nc.sync, nc.gpsimd, cen_p["c", 3])
        nc.sync.dma_start(out=wo_b, in_=io["Wo"].rearrange("(h p) d -> p h d", p=DH))

        # pre-stream PE: only what exp slot 0 needs
        ln_apply(cen_p["c", 0], gc, bc_, cnT, 0, 0, FC_C)
        k_unit(0, 0)
        ln_apply(cen_p["x", 0], gx, bx, xnT, 0, 0, FC_X)
        q_unit(0, 0)
        # exp table load: after every pre-stream Sqrt has been emitted.
        dummy = stat.tile([1, 1], f32, tag="dummy", bufs=1, name="dummy")
        nc.scalar.activation(out=dummy, in_=eps_t[0:1, 0:1], func=AF.Exp)

        gf_bc = bcast_load(io["lnf_g"], DQ, "gf_bc")
        bf_bc = bcast_load(io["lnf_b"], DQ, "bf_bc")
        bo_bc = bcast_load(io["bo"], DQ, "bo_bc")

        def y_init(t):
            nc.vector.tensor_add(out=y_sb[:, t], in0=xres[:, t], in1=bo_bc)

        # deferred units, emitted right after slot (c, qh, mc)'s
        # score+exp (and before that slot's PV pair).
        defer = {
            (c, qh): {mc: [] for mc in range(TK)}
            for c in range(IC)
            for qh in range(2)
        }

        def put(c, qh, mc, fn):
            defer[(c, qh)][mc].append(fn)

        # remaining LN applies + K(0, g) + V heads 0-3 inside (0, 0);
        # x group 1 + Q(0, th1) late in (0, 0) for sub-block (0, 1).
        for g in (1, 2, 3):
            lo = 2 * g - 1
            app = functools.partial(ln_apply, cen_p["c", g], gc, bc_, cnT, g)
            put(0, 0, lo, functools.partial(app, 0, 3))
            put(0, 0, lo + 1, functools.partial(app, 3, FC_C))
            put(0, 0, lo + 1, functools.partial(k_unit, 0, g))
        put(0, 0, 7, functools.partial(ln_apply, cen_p["x", 1], gx, bx, xnT, 1, 0, FC_X))
        put(0, 0, 9, functools.partial(q_unit, 0, 1))
        for t in range(TQ):
            put(0, 1, 8 + t, functools.partial(y_init, t))
        for mc in range(TK):
            put(0, 0, max(0, mc - 2), functools.partial(v_unit, 0, mc))
        # V heads 4-7 (first used at c=2) split over (1,0) and (1,1)
        for mc in range(TK):
            c, qh = (1, 0) if mc < 8 else (1, 1)
            put(c, qh, 2 * (mc % 8), functools.partial(v_unit, 1, mc))
        # K chunk c+1: cond groups 0/1 during (c, 1); groups 2/3 early in
        # (c+1, 0) (their key chunks aren't needed until slots 8/12).
        for c in range(IC - 1):
            put(c, 1, 1, functools.partial(k_unit, c + 1, 0))
            put(c, 1, 5, functools.partial(k_unit, c + 1, 1))
            put(c + 1, 0, 1, functools.partial(k_unit, c + 1, 2))
            put(c + 1, 0, 5, functools.partial(k_unit, c + 1, 3))
            put(c, 1, 3, functools.partial(q_unit, c + 1, 0))
            put(c, 1, 11, functools.partial(q_unit, c + 1, 1))
        # Wo pass for block c during sub-block (c+1, 0)
        for c in range(IC - 1):
            for t in range(TQ):
                put(c + 1, 0, 2 * t + 1, functools.partial(wo_unit, c, t))

        for c in range(IC):
            for qh in range(2):
                for mc in range(TK):
                    score_exp(c, mc, qh)
                    for fn in defer[(c, qh)][mc]:
                        fn()
                    if mc % 2 == 1:
                        pv_pair(c, (mc - 1) // 2)
                normalize(c, qh)

        # ---- tail: last Wo pass + batched final LayerNorm ---------------
        for t in range(TQ):
            wo_unit(IC - 1, t)
            stats = stat.tile([P, 6], f32, tag="bnstats", bufs=6, name="stf")
            nc.vector.bn_stats(out=stats, in_=y_sb[:, t])
            nc.vector.bn_aggr(out=mvs[:, t], in_=stats)
        stdf = stat.tile([P, TQ], f32, tag="stdf", bufs=1, name="stdf")
        for t in range(TQ):
            nc.scalar.activation(
                out=stdf[:, t : t + 1], in_=mvs[:, t, 1:2], func=AF.Sqrt, bias=eps_t
            )
        rstdf = stat.tile([P, TQ], f32, tag="rstdf", bufs=1, name="rstdf")
        nc.vector.reciprocal(out=rstdf, in_=stdf)
        for t in range(TQ):
            y1 = y_sb[:, t]
            nc.vector.tensor_scalar(
                out=y1,
                in0=y1,
                scalar1=mvs[:, t, 0:1],
                scalar2=rstdf[:, t : t + 1],
                op0=OP.subtract,
                op1=OP.mult,
            )
            nc.vector.tensor_mul(out=y1, in0=y1, in1=gf_bc)
            nc.gpsimd.tensor_add(out=y1, in0=y1, in1=bf_bc)
            (nc.sync if t % 2 == 0 else nc.gpsimd).dma_start(out=outr[:, t], in_=y1)


@functools.cache
def _build_program():
    import concourse.bacc as bacc
    import concourse.mybir as mybir
    import concourse.tile as tile

    f32 = mybir.dt.float32
    bf16 = mybir.dt.bfloat16
    nc = bacc.Bacc()
    io = {}
    io["x"] = nc.declare_dram_parameter("x", [NQ, DQ], f32, False)[:, :]
    io["cond"] = nc.declare_dram_parameter("cond", [M, DC], f32, False)[:, :]
    for name in ("lnx_g", "lnx_b"):
        io[name] = nc.declare_dram_parameter(name, [DQ], f32, False)[:]
    for name in ("lnc_g", "lnc_b"):
        io[name] = nc.declare_dram_parameter(name, [DC], f32, False)[:]
    wdt = mybir.dt.float8e4 if USE_DR_PROJ else bf16
    io["Wq"] = nc.declare_dram_parameter("Wq", [DQ, INNER], wdt, False)[:, :]
    io["Wk"] = nc.declare_dram_parameter("Wk", [DC, INNER], wdt, False)[:, :]
    io["Wv"] = nc.declare_dram_parameter("Wv", [DC, INNER], wdt, False)[:, :]
    io["Wo"] = nc.declare_dram_parameter("Wo", [INNER, DQ], bf16, False)[:, :]
    for name in ("bo", "lnf_g", "lnf_b"):
        io[name] = nc.declare_dram_parameter(name, [DQ], f32, False)[:]
    io["out"] = nc.declare_dram_parameter("out", [NQ, DQ], f32, True)[:, :]

    with tile.TileContext(nc) as tc:
        _emit(tc, io)
    nc.compile()
    return nc


def _core_input_map(inputs, core):
    import ml_dtypes

    b, half = core // 2, core % 2
    m = {
        "x": np.ascontiguousarray(inputs["x"][b, half * NQ : (half + 1) * NQ]),
        "cond": np.ascontiguousarray(inputs["cond"][b]),
    }
    for name in ("lnx_g", "lnx_b", "lnc_g", "lnc_b", "bo", "lnf_g", "lnf_b"):
        m[name] = np.asarray(inputs[name], dtype=np.float32)
    qkv_dt = ml_dtypes.float8_e4m3fn if USE_DR_PROJ else ml_dtypes.bfloat16
    for name in ("Wq", "Wk", "Wv"):
        m[name] = np.asarray(inputs[name]).astype(qkv_dt)
    m["Wo"] = np.asarray(inputs["Wo"]).astype(ml_dtypes.bfloat16)
    return m


TRACE = False
LAST_RESULTS = None


def kernel(**inputs):
    from concourse.bass_utils import run_bass_kernel_spmd

    global LAST_RESULTS
    nc = _build_program()
    in_maps = [_core_input_map(inputs, core) for core in range(N_CORES)]
    res = run_bass_kernel_spmd(
        nc,
        in_maps,
        list(range(N_CORES)),
        trace=TRACE,
        trace_cores=[0] if TRACE else None,
    )
    LAST_RESULTS = res
    out = np.empty((B, N, DQ), np.float32)
    for core in range(N_CORES):
        b, half = core // 2, core % 2
        out[b, half * NQ : (half + 1) * NQ] = res.results[core]["out"]
    return out


# revision 25
# speedup vs baseline: 1.2906x; 1.0584x over previous
"""Cross-attention block on 8 Trainium2 NeuronCores.

Computes, per batch b:
    xn = LN(x); cn = LN(cond)
    q = xn @ Wq; k = cn @ Wk; v = cn @ Wv   (8 heads x 64)
    out = softmax(q k^T / sqrt(64)) v
    y  = LN(out @ Wo + bo + x)

Sharding: 8 cores = 4 batches x 2 query-row halves (data parallel over
(batch, query-block)).  Each core recomputes LN(cond)/K/V for its batch
(duplicated across the 2 cores of a batch) and produces a disjoint
[1024, 512] slice of the output, so no collectives are needed.

v2 structure (vs the v1 baseline at 377us):
 - Phase A fuses LN -> PE-transpose -> Q/K/V projections per 512-token
   group, so the tensor engine has no idle gap longer than the HAM
   re-throttle window (3.4us) and runs at 2.4 GHz instead of 1.2.
 - Weights are cast to bf16 on the host, halving weight DMA and
   removing the on-device cast.
 - Score matmuls write bf16 directly to PSUM, so one exp ACTIVATE
   covers 2048 elements/partition (64 exps instead of 128).
 - PSUM ring: score tiles 2x2 banks + a shared 4x1-bank f32
   accumulator tag (projections, attention-out, Wo), sized to exactly
   8 banks; attention-out double buffering across head-pair blocks
   removes the per-block PE stall that re-throttled the clock.
 - Softmax denominators use reciprocal_approx_fast (~5x faster).
 - Wo + residual + LN stats for the first query half run inside the
   attention phase; only the final sqrt/scale runs as a tail.
"""

import functools

import numpy as np

B, N, M = 4, 2048, 2048
DQ, DC = 512, 768
H, DH = 8, 64
INNER = H * DH  # 512
P = 128
NQ = N // 2  # query rows per core
EPS = 1e-5
N_CORES = 8

FC_X = DQ // P  # 4 feature chunks of x
FC_C = DC // P  # 6 feature chunks of cond
IC = INNER // P  # 4 inner chunks
TQ = NQ // P  # 8 query-token chunks per core
TK = M // P  # 16 key-token chunks
NT = NQ // 512  # 2 query column tiles (transposed layout)
CG = M // 512  # 4 cond token groups


def _emit(tc, io):
    import contextlib

    import concourse.bass as bass
    import concourse.mybir as mybir

    nc = tc.nc
    f32 = mybir.dt.float32
    bf16 = mybir.dt.bfloat16
    AF = mybir.ActivationFunctionType
    OP = mybir.AluOpType

    ctx = contextlib.ExitStack()
    with ctx:
        singles = ctx.enter_context(tc.tile_pool(name="singles", bufs=1))
        work = ctx.enter_context(tc.tile_pool(name="work", bufs=3))
        stat = ctx.enter_context(tc.tile_pool(name="stat", bufs=4))
        cenp = ctx.enter_context(tc.tile_pool(name="cenp", bufs=5))
        ppool = ctx.enter_context(tc.tile_pool(name="ppool", bufs=3))
        ps = ctx.enter_context(tc.tile_pool(name="ps", bufs=2, space="PSUM"))

        # ---- constants -------------------------------------------------
        from concourse.masks import make_identity

        ident = singles.tile([P, P], bf16, name="ident")
        make_identity(nc, ident)
        eps_t = singles.tile([P, 1], f32, name="eps_t")
        nc.vector.memset(eps_t, EPS)

        def bcast_load(vec_ap, width, name):
            """[width] dram vector -> [128, width] sbuf tile (same row on
            every partition)."""
            t = singles.tile([P, width], f32, name=name)
            bc = bass.AP(
                tensor=vec_ap.tensor,
                offset=vec_ap.offset,
                ap=[[0, P]] + [list(a) for a in vec_ap.ap],
            )
            nc.gpsimd.dma_start(out=t, in_=bc)
            return t

        def strip_load(vec_ap, chunks, name):
            """[chunks*128] dram vector -> [128, chunks] sbuf (feature-on-
            partition layout)."""
            t = singles.tile([P, chunks], f32, name=name)
            nc.sync.dma_start(out=t, in_=vec_ap.rearrange("(c p) -> p c", p=P))
            return t

        gx = strip_load(io["lnx_g"], FC_X, "gx")
        bx = strip_load(io["lnx_b"], FC_X, "bx")
        gc = strip_load(io["lnc_g"], FC_C, "gc")
        bc_ = strip_load(io["lnc_b"], FC_C, "bc")

        # ---- weights: bf16 in HBM (host-cast), contraction on partitions.
        # DMAs for these are emitted inside the phase-A loop, ordered so the
        # first cond token group lands before the weights hog the queue.
        wq_b = singles.tile([P, FC_X, INNER], bf16, name="wq_b")
        wk_b = singles.tile([P, FC_C, INNER], bf16, name="wk_b")
        wv_b = singles.tile([P, FC_C, INNER], bf16, name="wv_b")
        # Wo in head-major rows to match the 64-partition O^T layout.
        wo_b = singles.tile([DH, H, DQ], bf16, name="wo_b")

        # ---- persistent activations ------------------------------------
        xnT = singles.tile([P, FC_X, NQ], bf16, name="xnT")  # LN(x)^T
        cnT = singles.tile([P, FC_C, M], bf16, name="cnT")  # LN(cond)^T
        QT = singles.tile([P, IC, NQ], bf16, name="QT")  # (q*scale)^T
        KT = singles.tile([P, IC, M], bf16, name="KT")  # k^T
        # v in token layout, one ones-column per head for the fused
        # softmax denominator: V_sb[:, mc, h, 0:64] = v, [..., 64] = 1.
        V_sb = singles.tile([P, TK, H, DH + 1], bf16, name="V_sb")
        nc.vector.memset(V_sb[:, :, :, DH : DH + 1], 1.0)
        # attn out^T, head-major on 64 partitions
        OT = singles.tile([DH, H, NQ], bf16, name="OT")
        # x residual tiles (bo gets folded in during phase B) and the
        # pre-normalize final output accumulator
        xres = singles.tile([P, TQ, DQ], f32, name="xres")
        xb = singles.tile([P, TQ, DQ], f32, name="xb")  # x + bo
        y1s = singles.tile([P, TQ, DQ], f32, name="y1s")
        mvs = singles.tile([P, TQ, 2], f32, name="mvs")

        # ---- phase A: LN + PE transpose + projections, per token group --
        # PSUM tags: "st" ([P,4,512] bf16 = 2 banks, 2 bufs) shared by the
        # phase-A transposes and the phase-B score tiles; "acc"
        # ([P,512] f32 = 1 bank, 4 bufs) shared by projection / attention-
        # out / Wo accumulators.  4 + 4 = 8 banks exactly.
        def ln_group(src, width, tg, g_strip, b_strip, dst):
            import math

            fmax = math.gcd(512, width)
            nsub = width // fmax
            fc_n = width // P
            cents, mvl = [], []
            std4 = stat.tile([P, 4], f32, tag="std", name="std4")
            for tl in range(4):
                t = tg * 4 + tl
                if src is None:  # x: already resident in xres
                    x_t = xres[:, t]
                else:
                    x_t = work.tile([P, width], f32, tag="xin", bufs=4, name="x_t")
                    nc.sync.dma_start(out=x_t, in_=src[:, t])
                if nsub == 1:
                    stats = stat.tile([P, 6], f32, tag="bnstats", bufs=6, name="st6")
                    nc.vector.bn_stats(out=stats, in_=x_t)
                else:
                    xr = x_t.rearrange("p (s f) -> p s f", f=fmax)
                    stats = stat.tile(
                        [P, nsub, 6], f32, tag="bnstats", bufs=6, name="st6"
                    )
                    for s in range(nsub):
                        nc.vector.bn_stats(out=stats[:, s], in_=xr[:, s])
                mv = stat.tile([P, 2], f32, tag="bnaggr", bufs=6, name="mv")
                nc.vector.bn_aggr(out=mv, in_=stats)
                nc.scalar.activation(
                    out=std4[:, tl : tl + 1],
                    in_=mv[:, 1:2],
                    func=AF.Sqrt,
                    bias=eps_t,
                    scale=1.0,
                )
                cents.append(x_t)
                mvl.append(mv)
            rstd = stat.tile([P, 4], f32, tag="rstd", name="rstd")
            nc.vector.reciprocal(out=rstd, in_=std4)
            nmr = stat.tile([P, 4], f32, tag="nmr", name="nmr")
            for tl in range(4):
                nc.vector.scalar_tensor_tensor(
                    out=nmr[:, tl : tl + 1],
                    in0=mvl[tl][:, 0:1],
                    scalar=-1.0,
                    in1=rstd[:, tl : tl + 1],
                    op0=OP.mult,
                    op1=OP.mult,
                )
            cen_ts = []
            for tl in range(4):
                cen = cenp.tile([P, width], bf16, tag="cen", name="cen")
                # (x - mean) * rstd on the scalar engine (idle in phase A)
                nc.scalar.activation(
                    out=cen,
                    in_=cents[tl],
                    func=AF.Identity,
                    bias=nmr[:, tl : tl + 1],
                    scale=rstd[:, tl : tl + 1],
                )
                cen_ts.append(cen)
            for fc in range(fc_n):
                tp = ps.tile([P, 4, P], bf16, tag="st", bufs=2, name="tp")
                for tl in range(4):
                    nc.tensor.transpose(
                        tp[:, tl], cen_ts[tl][:, fc * P : (fc + 1) * P], ident
                    )
                # dst = tp * g[fc] + b[fc]   (per-partition scalars)
                nc.vector.tensor_scalar(
                    out=dst[:, fc, tg * 512 : (tg + 1) * 512],
                    in0=tp,
                    scalar1=g_strip[:, fc : fc + 1],
                    scalar2=b_strip[:, fc : fc + 1],
                    op0=OP.mult,
                    op1=OP.add,
                )

        condr = io["cond"].rearrange("(t p) d -> p t d", p=P)
        xr = io["x"].rearrange("(t p) d -> p t d", p=P)

        scale = float(DH) ** -0.5
        for tg in range(CG):
            ln_group(condr, DC, tg, gc, bc_, cnT)
            if tg == 0:
                # Queue the weight loads behind the first cond group.
                nc.sync.dma_start(
                    out=wk_b, in_=io["Wk"].rearrange("(ko p) i -> p ko i", p=P)
                )
                nc.sync.dma_start(
                    out=wv_b, in_=io["Wv"].rearrange("(ko p) i -> p ko i", p=P)
                )
            if tg == 1:
                nc.sync.dma_start(
                    out=wq_b, in_=io["Wq"].rearrange("(ko p) i -> p ko i", p=P)
                )
                nc.sync.dma_start(
                    out=wo_b, in_=io["Wo"].rearrange("(h p) d -> p h d", p=DH)
                )
                # x tiles land in their long-lived residual slots.
                for t in range(TQ):
                    nc.sync.dma_start(out=xres[:, t], in_=xr[:, t])
            sl = slice(tg * 512, (tg + 1) * 512)
            # KT columns for this token group
            for m in range(IC):
                kps = ps.tile([P, 512], f32, tag="acc", bufs=4, name="kps")
                for k in range(FC_C):
                    nc.tensor.matmul(
                        kps,
                        lhsT=wk_b[:, k, m * P : (m + 1) * P],
                        rhs=cnT[:, k, sl],
                        start=(k == 0),
                        stop=(k == FC_C - 1),
                    )
                nc.scalar.copy(out=KT[:, m, sl], in_=kps)
            # V rows for this token group
            for mc in range(tg * 4, tg * 4 + 4):
                vps = ps.tile([P, 512], f32, tag="acc", bufs=4, name="vps")
                for k in range(FC_C):
                    nc.tensor.matmul(
                        vps,
                        lhsT=cnT[:, k, mc * P : (mc + 1) * P],
                        rhs=wv_b[:, k, :],
                        start=(k == 0),
                        stop=(k == FC_C - 1),
                    )
                nc.scalar.copy(
                    out=V_sb[:, mc, :, 0:DH],
                    in_=vps.rearrange("p (h d) -> p h d", h=H),
                )

        gf_bc = bcast_load(io["lnf_g"], DQ, "gf_bc")
        bf_bc = bcast_load(io["lnf_b"], DQ, "bf_bc")
        bo_bc = bcast_load(io["bo"], DQ, "bo_bc")

        for xg in range(NT):
            ln_group(None, DQ, xg, gx, bx, xnT)
            sl = slice(xg * 512, (xg + 1) * 512)
            for m in range(IC):
                qps = ps.tile([P, 512], f32, tag="acc", bufs=4, name="qps")
                for k in range(FC_X):
                    nc.tensor.matmul(
                        qps,
                        lhsT=wq_b[:, k, m * P : (m + 1) * P],
                        rhs=xnT[:, k, sl],
                        start=(k == 0),
                        stop=(k == FC_X - 1),
                    )
                nc.scalar.activation(
                    out=QT[:, m, sl], in_=qps, func=AF.Copy, scale=scale
                )

        # Fold bo into the residual tiles (gpsimd; runs during phase B).
        for t in range(TQ):
            nc.gpsimd.tensor_add(out=xb[:, t], in0=xres[:, t], in1=bo_bc)

        # Dummy exp: pulls the exp table-set load off phase B's critical path.
        dummy = stat.tile([1, 1], f32, tag="dummy", bufs=1, name="dummy")
        nc.scalar.activation(out=dummy, in_=eps_t[0:1, 0:1], func=AF.Exp)

        # ---- phase B/C: attention + Wo/residual per query tile ----------
        def emit_block(nt, c):
            hA, hB = 2 * c, 2 * c + 1
            ot = {
                h: ps.tile([P, 512], f32, tag="acc", bufs=4, name=f"ot{h % 2}")
                for h in (hA, hB)
            }
            sl = slice(nt * 512, (nt + 1) * 512)
            q_a = QT[0:DH, c, sl]
            q_b = QT[DH:P, c, sl]

            def emit_pv(mc, p):
                for j, h in enumerate((hA, hB)):
                    nc.tensor.matmul(
                        ot[h][0 : DH + 1, :],
                        lhsT=V_sb[:, mc, h, :],
                        rhs=p[:, j],
                        start=(mc == 0),
                        stop=(mc == TK - 1),
                    )

            # st slot j = head j, so consecutive score matmuls alternate
            # PE row groups (h0 / h64) and LDWEIGHTS pulls ahead.
            pend = None
            for mc in range(TK):
                st = ps.tile([P, 2, 512], f32, tag="st", bufs=2, name="stb")
                nc.tensor.matmul(
                    st[:, 0],
                    lhsT=KT[0:DH, c, mc * P : (mc + 1) * P],
                    rhs=q_a,
                    start=True,
                    stop=True,
                )
                nc.tensor.matmul(
                    st[:, 1],
                    lhsT=KT[DH:P, c, mc * P : (mc + 1) * P],
                    rhs=q_b,
                    start=True,
                    stop=True,
                )
                p = ppool.tile([P, 2, 512], bf16, tag="p", name="p")
                nc.scalar.activation(out=p, in_=st, func=AF.Exp)
                if pend is not None:
                    emit_pv(*pend)
                pend = (mc, p)
            emit_pv(*pend)

            # normalize: row DH of ot[h] holds the softmax denominator.
            for h in (hA, hB):
                rb = work.tile([P, 512], f32, tag="rb", bufs=2, name="rb")
                # full-tile approx reciprocal (5x faster; the sliced form is
                # broken, the full-tile form is verified).  Only partition 64
                # (the denominator row) is meaningful; junk partitions are
                # overwritten by the broadcast before the only read.
                nc.vector.reciprocal_approx_fast(out=rb, in_=ot[h])
                r0 = work.tile([1, 512], f32, tag="r0", bufs=2, name="r0")
                nc.sync.dma_start(out=r0, in_=rb[DH : DH + 1, :])
                nc.gpsimd.partition_broadcast(rb[0:DH, :], r0[0:1, :])
                nc.vector.tensor_mul(
                    out=OT[:, h, nt * 512 : (nt + 1) * 512],
                    in0=ot[h][0:DH, :],
                    in1=rb[0:DH, :],
                )

        def emit_wo(ts):
            # Wo + residual + LN stats for these query tiles; the sqrt and
            # final scale run later so the exp table set stays loaded.
            for t in ts:
                y_ps = ps.tile([P, 512], f32, tag="acc", bufs=4, name="y_ps")
                for h in range(H):
                    nc.tensor.matmul(
                        y_ps,
                        lhsT=OT[:, h, t * P : (t + 1) * P],
                        rhs=wo_b[:, h, :],
                        start=(h == 0),
                        stop=(h == H - 1),
                    )
                nc.vector.tensor_add(out=y1s[:, t], in0=y_ps, in1=xb[:, t])
                stats = stat.tile([P, 6], f32, tag="bnstats", bufs=6, name="stf")
                nc.vector.bn_stats(out=stats, in_=y1s[:, t])
                nc.vector.bn_aggr(out=mvs[:, t], in_=stats)

        outr = io["out"].rearrange("(t p) d -> p t d", p=P)

        def final_apply(ts):
            # sqrt + scale + store for a set of query tiles (their Wo /
            # residual / stats must already be emitted).
            n = len(ts)
            stdn = stat.tile([P, n], f32, tag="stdn", bufs=2, name="stdn")
            for i, t in enumerate(ts):
                nc.scalar.activation(
                    out=stdn[:, i : i + 1],
                    in_=mvs[:, t, 1:2],
                    func=AF.Sqrt,
                    bias=eps_t,
                    scale=1.0,
                )
            rstdn = stat.tile([P, n], f32, tag="rstdn", bufs=2, name="rstdn")
            nc.vector.reciprocal(out=rstdn, in_=stdn)
            for i, t in enumerate(ts):
                y1 = y1s[:, t]
                nc.vector.tensor_scalar(
                    out=y1,
                    in0=y1,
                    scalar1=mvs[:, t, 0:1],
                    scalar2=rstdn[:, i : i + 1],
                    op0=OP.subtract,
                    op1=OP.mult,
                )
                nc.vector.tensor_mul(out=y1, in0=y1, in1=gf_bc)
                nc.gpsimd.tensor_add(out=y1, in0=y1, in1=bf_bc)
                nc.sync.dma_start(out=outr[:, t], in_=y1)

        # nt=0's Wo/residual work is spread across nt=1's blocks (one or
        # two query tiles per boundary, so the shared PSUM accumulator ring
        # never makes a Wo matmul wait on a fresh block's normalize); its
        # final LN apply (one sqrt table switch) hides under later blocks'
        # exps.
        blocks = [(nt, c) for nt in range(NT) for c in range(H // 2)]
        for i, (nt, c) in enumerate(blocks):
            emit_block(nt, c)
            if (nt, c) == (1, 0):
                emit_wo([0, 1, 2, 3])
            elif (nt, c) == (1, 2):
                final_apply([0, 1, 2, 3])
        emit_wo([4, 5, 6, 7])
        final_apply([4, 5, 6, 7])

        if "p_cnT" in io:
            nc.sync.dma_start(out=io["p_cnT"], in_=cnT)
            nc.sync.dma_start(out=io["p_xnT"], in_=xnT)
            nc.sync.dma_start(out=io["p_QT"], in_=QT)
            nc.sync.dma_start(out=io["p_KT"], in_=KT)
            nc.sync.dma_start(out=io["p_V"], in_=V_sb)
            nc.sync.dma_start(out=io["p_OT"], in_=OT)



@functools.cache
def _build_program():
    global PROBE
    import concourse.bacc as bacc
    import concourse.mybir as mybir
    import concourse.tile as tile

    f32 = mybir.dt.float32
    bf16 = mybir.dt.bfloat16
    nc = bacc.Bacc()
    io = {}
    io["x"] = nc.declare_dram_parameter("x", [NQ, DQ], f32, False)[:, :]
    io["cond"] = nc.declare_dram_parameter("cond", [M, DC], f32, False)[:, :]
    for name in ("lnx_g", "lnx_b"):
        io[name] = nc.declare_dram_parameter(name, [DQ], f32, False)[:]
    for name in ("lnc_g", "lnc_b"):
        io[name] = nc.declare_dram_parameter(name, [DC], f32, False)[:]
    io["Wq"] = nc.declare_dram_parameter("Wq", [DQ, INNER], bf16, False)[:, :]
    io["Wk"] = nc.declare_dram_parameter("Wk", [DC, INNER], bf16, False)[:, :]
    io["Wv"] = nc.declare_dram_parameter("Wv", [DC, INNER], bf16, False)[:, :]
    io["Wo"] = nc.declare_dram_parameter("Wo", [INNER, DQ], bf16, False)[:, :]
    for name in ("bo", "lnf_g", "lnf_b"):
        io[name] = nc.declare_dram_parameter(name, [DQ], f32, False)[:]
    io["out"] = nc.declare_dram_parameter("out", [NQ, DQ], f32, True)[:, :]
    if PROBE:
        bf = bf16
        io["p_cnT"] = nc.declare_dram_parameter("p_cnT", [P, FC_C, M], bf, True)[:, :, :]
        io["p_xnT"] = nc.declare_dram_parameter("p_xnT", [P, FC_X, NQ], bf, True)[:, :, :]
        io["p_QT"] = nc.declare_dram_parameter("p_QT", [P, IC, NQ], bf, True)[:, :, :]
        io["p_KT"] = nc.declare_dram_parameter("p_KT", [P, IC, M], bf, True)[:, :, :]
        io["p_V"] = nc.declare_dram_parameter("p_V", [P, TK, H, DH + 1], bf, True)[:, :, :, :]
        io["p_OT"] = nc.declare_dram_parameter("p_OT", [DH, H, NQ], bf, True)[:, :, :]

    with tile.TileContext(nc) as tc:
        _emit(tc, io)
    nc.compile()
    return nc


def _core_input_map(inputs, core):
    import ml_dtypes

    b, half = core // 2, core % 2
    m = {
        "x": np.ascontiguousarray(inputs["x"][b, half * NQ : (half + 1) * NQ]),
        "cond": np.ascontiguousarray(inputs["cond"][b]),
    }
    for name in ("lnx_g", "lnx_b", "lnc_g", "lnc_b", "bo", "lnf_g", "lnf_b"):
        m[name] = np.asarray(inputs[name], dtype=np.float32)
    for name in ("Wq", "Wk", "Wv", "Wo"):
        m[name] = np.asarray(inputs[name]).astype(ml_dtypes.bfloat16)
    return m


TRACE = False
PROBE = False
LAST_RESULTS = None


def kernel(**inputs):
    from concourse.bass_utils import run_bass_kernel_spmd

    global LAST_RESULTS
    nc = _build_program()
    in_maps = [_core_input_map(inputs, core) for core in range(N_CORES)]
    res = run_bass_kernel_spmd(
        nc,
        in_maps,
        list(range(N_CORES)),
        trace=TRACE,
        trace_cores=[0] if TRACE else None,
    )
    LAST_RESULTS = res
    out = np.empty((B, N, DQ), np.float32)
    for core in range(N_CORES):
        b, half = core // 2, core % 2
        out[b, half * NQ : (half + 1) * NQ] = res.results[core]["out"]
    return out

